# revision 1
# baseline (speedup 1.0000x reference)
"""Multi-head attention (B=2, L=2048, D=1024, H=16) on 8 TRN2 NeuronCores.

Sharding: core c handles batch b = c//4 and head group g = c%4 (4 heads,
256 features). Each core:
  - projects q, k (feature-major) and v (row-major, with a ones-column per
    head so the attn@V matmul emits softmax denominators for free)
  - computes scores^T = K_h Q_h^T tile-wise (keys on partitions), exp on
    ScalarE with the 1/sqrt(DH) scale folded in (no max subtraction: scores
    are ~N(0,1), exp is safe in fp32)
  - accumulates ctx^T = V_h^T P_h in PSUM, normalizes by the denominator row
    via a K=1 broadcast matmul + DVE multiply
  - computes its partial of the output projection out^T = Wo_h ctx^T
The host sums the 4 per-head-group partials per batch and adds bo.
No inter-core communication.
"""

import math
import os
import sys

sys.path.insert(0, "/opt/trn_rl_repo")

import ml_dtypes
import numpy as np

import concourse.bass as bass
import concourse.mybir as mybir
import concourse.tile as tile
from concourse import bacc
from concourse.bass_utils import run_bass_kernel_spmd

B, L, D, H, DH = 2, 2048, 1024, 16, 64
NCORES = 8
HPC = 4                  # heads per core
FPC = HPC * DH           # 256 features per core
ND = D // 128            # 8 contraction tiles
NFT = FPC // 128         # 2 feature tiles for q/k/ctx
NM = L // 128            # 16 key tiles
VW = DH + 1              # 65 = head block width in v (64 feats + ones col)
VROW = HPC * VW          # 260
SCALE = 1.0 / math.sqrt(DH)
CDT = mybir.dt.bfloat16
NP_CDT = ml_dtypes.bfloat16
F32 = mybir.dt.float32
EXP = mybir.ActivationFunctionType.Exp
OUT_NAME = "outT"

_CACHE = {}


def build_nc():
    nc = bacc.Bacc(
        "TRN2",
        target_bir_lowering=False,
        debug=False,
        enable_asserts=False,
        num_devices=NCORES,
    )
    xqT_d = nc.dram_tensor("xqT", [D, L], CDT, kind="ExternalInput")
    xkT_d = nc.dram_tensor("xkT", [D, L], CDT, kind="ExternalInput")
    xvT_d = nc.dram_tensor("xvT", [D, L], CDT, kind="ExternalInput")
    wq_d = nc.dram_tensor("wqT", [D, FPC], CDT, kind="ExternalInput")
    wk_d = nc.dram_tensor("wkT", [D, FPC], CDT, kind="ExternalInput")
    wv_d = nc.dram_tensor("wvT", [D, VROW], CDT, kind="ExternalInput")
    wo_d = nc.dram_tensor("woT", [FPC, D], CDT, kind="ExternalInput")
    bq_d = nc.dram_tensor("bq2", [128, NFT], F32, kind="ExternalInput")
    bk_d = nc.dram_tensor("bk2", [128, NFT], F32, kind="ExternalInput")
    bvb_d = nc.dram_tensor("bvb", [128, VROW], F32, kind="ExternalInput")
    out_d = nc.dram_tensor(OUT_NAME, [D, L], F32, kind="ExternalOutput")

    with tile.TileContext(nc) as tc:
        with tc.tile_pool(name="persist", bufs=1) as pp:
            qT = pp.tile([128, NFT, L], CDT)
            kT = pp.tile([128, NFT, L], CDT)
            vsb = pp.tile([128, NM, VROW], CDT)
            ctxT = pp.tile([128, NFT, L], CDT)
            wo_sb = pp.tile([128, NFT, D], CDT)
            bq_sb = pp.tile([128, NFT], F32)
            bk_sb = pp.tile([128, NFT], F32)
            bvb_sb = pp.tile([128, VROW], F32)
            ones_sb = pp.tile([1, 128], F32)

            # keep the Sync DGE queue clear for the projection inputs the
            # first matmuls need: route weights/biases not needed until much
            # later through the otherwise-idle GpSimd DGE
            nc.vector.memset(ones_sb[:], 1.0)
            nc.gpsimd.dma_start(bq_sb[:], bq_d[:])
            nc.gpsimd.dma_start(bk_sb[:], bk_d[:])
            nc.gpsimd.dma_start(bvb_sb[:], bvb_d[:])
            nc.gpsimd.dma_start(
                wo_sb[:], wo_d.rearrange("(n p) f -> p n f", p=128)
            )

            # ---- Phase A: projections ----
            with (
                tc.tile_pool(name="phA", bufs=1) as pa,
                tc.tile_pool(name="psA", bufs=4, space="PSUM") as psA,
            ):
                wq_sb = pa.tile([128, ND, FPC], CDT)
                wk_sb = pa.tile([128, ND, FPC], CDT)
                wv_sb = pa.tile([128, ND, VROW], CDT)
                xq_sb = pa.tile([128, ND, L], CDT)
                xk_sb = pa.tile([128, ND, L], CDT)
                xv_sb = pa.tile([128, ND, L], CDT)
                xq_r = xqT_d.rearrange("(n p) l -> p n l", p=128)
                xk_r = xkT_d.rearrange("(n p) l -> p n l", p=128)
                xv_r = xvT_d.rearrange("(n p) l -> p n l", p=128)
                wq_r = wq_d.rearrange("(n p) f -> p n f", p=128)
                wk_r = wk_d.rearrange("(n p) f -> p n f", p=128)
                wv_r = wv_d.rearrange("(n p) f -> p n f", p=128)
                for d in range(ND):
                    nc.sync.dma_start(wq_sb[:, d, :], wq_r[:, d, :])
                    nc.sync.dma_start(xq_sb[:, d, :], xq_r[:, d, :])
                for d in range(ND):
                    nc.sync.dma_start(wv_sb[:, d, :], wv_r[:, d, :])
                    nc.sync.dma_start(xv_sb[:, d, :], xv_r[:, d, :])
                for d in range(ND):
                    nc.sync.dma_start(wk_sb[:, d, :], wk_r[:, d, :])
                    nc.sync.dma_start(xk_sb[:, d, :], xk_r[:, d, :])
                # q/k projections d-outer: first matmul only needs d-tile 0
                # (fast ramp, PE warms early); 4 row-chunks share one lhsT
                def proj_qk(x_sb, w_sb, b_sb, dstT, ti):
                    for ft in range(NFT):
                        pss = [
                            psA.tile(
                                [128, 512], F32, tag="projqk",
                                name=f"pjk_{ti}_{ft}_{ch}",
                            )
                            for ch in range(4)
                        ]
                        for d in range(ND):
                            for ch in range(4):
                                nc.tensor.matmul(
                                    pss[ch][:],
                                    w_sb[:, d, ft * 128 : (ft + 1) * 128],
                                    x_sb[:, d, ch * 512 : (ch + 1) * 512],
                                    start=(d == 0),
                                    stop=(d == ND - 1),
                                )
                        for ch in range(4):
                            nc.vector.tensor_scalar_add(
                                dstT[:, ft, ch * 512 : (ch + 1) * 512],
                                pss[ch][:],
                                b_sb[:, ft : ft + 1],
                            )

                proj_qk(xq_sb, wq_sb, bq_sb, qT, 0)
                # v projection: row-major output [keys, feats+ones]
                for kt in range(NM):
                    ps = psA.tile([128, VROW], F32, tag="projv")
                    for d in range(ND):
                        nc.tensor.matmul(
                            ps[:],
                            xv_sb[:, d, kt * 128 : (kt + 1) * 128],
                            wv_sb[:, d, :],
                            start=(d == 0),
                            stop=(d == ND - 1),
                        )
                    nc.vector.tensor_add(vsb[:, kt, :], ps[:], bvb_sb[:])
                # k last: the A->B pool-boundary release then coincides with
                # kT readiness, which the first scores matmuls need anyway
                proj_qk(xk_sb, wk_sb, bk_sb, kT, 1)

            kphase = os.environ.get("KPHASE", "full")
            if kphase == "A":
                with tc.tile_pool(name="dbg", bufs=2) as dbg:
                    for ft in range(NFT):
                        st = dbg.tile([128, L], F32, tag="st")
                        nc.vector.tensor_copy(st[:], qT[:, ft, :])
                        nc.sync.dma_start(out_d[ft * 128 : (ft + 1) * 128, :], st[:])
                        st2 = dbg.tile([128, L], F32, tag="st")
                        nc.vector.tensor_copy(st2[:], kT[:, ft, :])
                        nc.sync.dma_start(
                            out_d[(2 + ft) * 128 : (3 + ft) * 128, :], st2[:]
                        )
                        st3 = dbg.tile([128, L], F32, tag="st")
                        nc.vector.tensor_copy(
                            st3[:, : 7 * VROW],
                            vsb[:, ft * 8 : ft * 8 + 7, :],
                        )
                        nc.vector.memset(st3[:, 7 * VROW :], 0.0)
                        nc.sync.dma_start(
                            out_d[(4 + ft) * 128 : (5 + ft) * 128, :], st3[:]
                        )
            # ---- Phase B: attention ----
            if kphase in ("AB", "full"):
                phase_bc(nc, tc, qT, kT, vsb, ctxT, wo_sb, ones_sb, out_d, kphase)
    nc.compile()
    return nc


def phase_bc(nc, tc, qT, kT, vsb, ctxT, wo_sb, ones_sb, out_d, kphase):
    with (
        tc.tile_pool(name="probs", bufs=48) as pb,
        tc.tile_pool(name="smalls", bufs=2) as sm,
        tc.tile_pool(name="psS", bufs=2, space="PSUM") as psS,
        tc.tile_pool(name="psC", bufs=2, space="PSUM") as psC,
        tc.tile_pool(name="psX", bufs=2, space="PSUM") as psX,
    ):
        def normalize(ctx, hi, hp, lc):
            # approx_fast mis-executes on HW when operands sit at base_partition
            # != 0, so stage the denominator row at partition 0 first
            den = sm.tile([1, 512], F32, tag="den", name=f"den_{hp}_{hi}_{lc}")
            nc.vector.tensor_copy(den[:], ctx[64:65, :])
            rec = sm.tile([1, 512], F32, tag="rec", name=f"rec_{hp}_{hi}_{lc}")
            nc.vector.reciprocal_approx_fast(rec[:], den[:])
            rb_ps = psX.tile([128, 512], F32, tag="acc512", name=f"rb_{hp}_{hi}_{lc}")
            nc.tensor.matmul(rb_ps[:], ones_sb[:], rec[:], start=True, stop=True)
            rb_sb = sm.tile([128, 512], F32, tag="rbsb", name=f"rbsb_{hp}_{hi}_{lc}")
            nc.vector.tensor_copy(rb_sb[:], rb_ps[:])
            po = hi * 64
            nc.vector.tensor_mul(
                ctxT[po : po + 64, hp, lc * 512 : (lc + 1) * 512],
                ctx[0:64, :],
                rb_sb[0:64, :],
            )

        for qh in range(2):
            for hp in range(HPC // 2):
                probs = {}
                # first-half attnV chains run inside the m loop, one per head
                chain = {
                    hi: psC.tile([VW, 512], F32, tag="ctx", name=f"ctx_{qh}_{hp}_{hi}_a")
                    for hi in range(2)
                }
                for m in range(NM):
                    scs = []
                    for hi in range(2):
                        po = hi * 64
                        sc = psS.tile([128, 1024], F32, tag="sc", name=f"sc_{qh}_{hp}_{m}_{hi}")
                        for c2 in range(2):
                            qo = qh * 1024 + c2 * 512
                            nc.tensor.matmul(
                                sc[:, c2 * 512 : (c2 + 1) * 512],
                                kT[po : po + 64, hp, m * 128 : (m + 1) * 128],
                                qT[po : po + 64, hp, qo : qo + 512],
                                start=True,
                                stop=True,
                            )
                        scs.append(sc)
                    for hi, sc in enumerate(scs):
                        pr = pb.tile([128, 1024], CDT, tag="probs", name=f"pr_{qh}_{hp}_{m}_{hi}")
                        nc.scalar.activation(pr[:], sc[:], EXP, scale=SCALE)
                        probs[(hi, m)] = pr
                        h = 2 * hp + hi
                        nc.tensor.matmul(
                            chain[hi][:],
                            vsb[:, m, h * VW : (h + 1) * VW],
                            pr[:, 0:512],
                            start=(m == 0),
                            stop=(m == NM - 1),
                        )
                for hi in range(2):
                    normalize(chain[hi], hi, hp, qh * 2)
                # second-half chains stream from fully materialized probs
                for hi in range(2):
                    h = 2 * hp + hi
                    ctx = psC.tile([VW, 512], F32, tag="ctx", name=f"ctx_{qh}_{hp}_{hi}_b")
                    for m in range(NM):
                        nc.tensor.matmul(
                            ctx[:],
                            vsb[:, m, h * VW : (h + 1) * VW],
                            probs[(hi, m)][:, 512:1024],
                            start=(m == 0),
                            stop=(m == NM - 1),
                        )
                    normalize(ctx, hi, hp, qh * 2 + 1)

            if kphase == "AB":
                continue
            # ---- output projection for the two finished q chunks ----
            for lc in (qh * 2, qh * 2 + 1):
                for ft8 in range(D // 128):
                    ops = psX.tile([128, 512], F32, tag="acc512", name=f"op_{lc}_{ft8}")
                    for d2 in range(NFT):
                        nc.tensor.matmul(
                            ops[:],
                            wo_sb[:, d2, ft8 * 128 : (ft8 + 1) * 128],
                            ctxT[:, d2, lc * 512 : (lc + 1) * 512],
                            start=(d2 == 0),
                            stop=(d2 == NFT - 1),
                        )
                    st = sm.tile([128, 512], F32, tag="ost", bufs=4, name=f"st_{lc}_{ft8}")
                    nc.vector.tensor_copy(st[:], ops[:])
                    nc.gpsimd.dma_start(
                        out_d[ft8 * 128 : (ft8 + 1) * 128, lc * 512 : (lc + 1) * 512],
                        st[:],
                    )

        if kphase == "AB":
            for ft in range(NFT):
                st = sm.tile([128, L], F32, tag="ostage", name=f"dbg_{ft}")
                nc.vector.tensor_copy(st[:], ctxT[:, ft, :])
                nc.sync.dma_start(out_d[ft * 128 : (ft + 1) * 128, :], st[:])


def make_in_maps(Q, K, V, Wq, bq, Wk, bk, Wv, bv, Wo, bo):
    Q = np.asarray(Q, np.float32)
    K = np.asarray(K, np.float32)
    V = np.asarray(V, np.float32)
    xqT = [np.ascontiguousarray(Q[b].T).astype(NP_CDT) for b in range(B)]
    xkT = [np.ascontiguousarray(K[b].T).astype(NP_CDT) for b in range(B)]
    xvT = [np.ascontiguousarray(V[b].T).astype(NP_CDT) for b in range(B)]
    in_maps = []
    for c in range(NCORES):
        b, g = divmod(c, HPC)
        fs = slice(g * FPC, (g + 1) * FPC)
        wqT = np.ascontiguousarray(np.asarray(Wq, np.float32)[fs, :].T).astype(NP_CDT)
        wkT = np.ascontiguousarray(np.asarray(Wk, np.float32)[fs, :].T).astype(NP_CDT)
        # v weights: per-head [64 cols | zero col], bias bcast carries the 1.0
        wv_blk = np.zeros((D, VROW), np.float32)
        bv_blk = np.zeros((VROW,), np.float32)
        wv_slc = np.asarray(Wv, np.float32)[fs, :].T  # [D, 256]
        bv_slc = np.asarray(bv, np.float32)[fs]
        for h in range(HPC):
            wv_blk[:, h * VW : h * VW + DH] = wv_slc[:, h * DH : (h + 1) * DH]
            bv_blk[h * VW : h * VW + DH] = bv_slc[h * DH : (h + 1) * DH]
            bv_blk[h * VW + DH] = 1.0
        woT = np.ascontiguousarray(np.asarray(Wo, np.float32)[:, fs].T).astype(NP_CDT)
        bq2 = np.ascontiguousarray(
            np.asarray(bq, np.float32)[fs].reshape(NFT, 128).T
        )
        bk2 = np.ascontiguousarray(
            np.asarray(bk, np.float32)[fs].reshape(NFT, 128).T
        )
        in_maps.append(
            {
                "xqT": xqT[b],
                "xkT": xkT[b],
                "xvT": xvT[b],
                "wqT": wqT,
                "wkT": wkT,
                "wvT": wv_blk.astype(NP_CDT),
                "woT": woT,
                "bq2": bq2,
                "bk2": bk2,
                "bvb": np.broadcast_to(bv_blk, (128, VROW)).copy(),
            }
        )
    return in_maps


def assemble(results, bo):
    out = np.zeros((B, L, D), np.float32)
    for c in range(NCORES):
        b = c // HPC
        out[b] += results[c][OUT_NAME].T
    out += np.asarray(bo, np.float32)[None, None, :]
    return out


def kernel(Q, K, V, Wq, bq, Wk, bk, Wv, bv, Wo, bo):
    if "nc" not in _CACHE:
        _CACHE["nc"] = build_nc()
    nc = _CACHE["nc"]
    in_maps = make_in_maps(Q, K, V, Wq, bq, Wk, bk, Wv, bv, Wo, bo)
    res = run_bass_kernel_spmd(nc, in_maps, core_ids=list(range(NCORES)))
    return assemble(res.results, bo)



# revision 2
# speedup vs baseline: 1.2495x; 1.2495x over previous
"""Multi-head attention (B=2, L=2048, D=1024, H=16) on 8 TRN2 NeuronCores.

Sharding: core c handles batch b = c//4 and head group g = c%4 (4 heads,
256 features). No inter-core communication; host sums the 4 per-head-group
output partials per batch and adds bo.

Per-core schedule (engine-balanced software pipeline):
  - warmup matmuls on a memset tile cover the ~9us framework/DMA startup and
    hold the PE HAM clock-gate at 2.4GHz; a tiny exp() preloads the ACT table
  - k then q projections (feature-major, bf16), v projection row-major with a
    ones-column per head so the attn@V chain emits softmax denominators free
  - attention runs as 8 half-blocks (512 queries x head-pair). Per key tile m:
    the two heads' scores matmuls (K=64) issue back-to-back at PE row groups
    (0,0)/(64,0) so they stream concurrently; one [128,1024] exp on ScalarE;
    two accumulating attn@V chain matmuls. The v projection is interleaved
    with the first block's scores so ScalarE starts exp'ing ~28us in and the
    pipeline is ScalarE-rate-limited (the sc PSUM ring paces the PE).
  - normalization: DVE reciprocal + GpSimd partition_broadcast + DVE multiply
    (no PE broadcast matmuls), then the output projection per query block.
"""

import math
import sys

sys.path.insert(0, "/opt/trn_rl_repo")

import ml_dtypes
import numpy as np

import concourse.bass as bass
import concourse.mybir as mybir
import concourse.tile as tile
from concourse import bacc
from concourse.bass_utils import run_bass_kernel_spmd

B, L, D, H, DH = 2, 2048, 1024, 16, 64
NCORES = 8
HPC = 4                  # heads per core
FPC = HPC * DH           # 256 features per core
ND = D // 128            # 8 contraction tiles
NFT = FPC // 128         # 2 feature tiles for q/k/ctx
NM = L // 128            # 16 key tiles
VW = DH + 1              # 65 = head block width in v (64 feats + ones col)
VROW = HPC * VW          # 260
NQB = 4                  # 512-query blocks
SCALE = 1.0 / math.sqrt(DH)
CDT = mybir.dt.bfloat16
NP_CDT = ml_dtypes.bfloat16
F32 = mybir.dt.float32
EXP = mybir.ActivationFunctionType.Exp
OUT_NAME = "outT"
# half-blocks: (query block, head pair)
HB = [(qb, hp) for qb in range(NQB) for hp in range(2)]

_CACHE = {}


def build_nc():
    nc = bacc.Bacc(
        "TRN2",
        target_bir_lowering=False,
        debug=False,
        enable_asserts=False,
        num_devices=NCORES,
    )
    xqT_d = nc.dram_tensor("xqT", [D, L], CDT, kind="ExternalInput")
    xkT_d = nc.dram_tensor("xkT", [D, L], CDT, kind="ExternalInput")
    xvT_d = nc.dram_tensor("xvT", [D, L], CDT, kind="ExternalInput")
    wq_d = nc.dram_tensor("wqT", [D, FPC], CDT, kind="ExternalInput")
    wk_d = nc.dram_tensor("wkT", [D, FPC], CDT, kind="ExternalInput")
    wv_d = nc.dram_tensor("wvT", [D, VROW], CDT, kind="ExternalInput")
    wo_d = nc.dram_tensor("woT", [FPC, D], CDT, kind="ExternalInput")
    bq_d = nc.dram_tensor("bq2", [128, NFT], F32, kind="ExternalInput")
    bk_d = nc.dram_tensor("bk2", [128, NFT], F32, kind="ExternalInput")
    bvb_d = nc.dram_tensor("bvb", [128, VROW], F32, kind="ExternalInput")
    out_d = nc.dram_tensor(OUT_NAME, [D, L], CDT, kind="ExternalOutput")

    with tile.TileContext(nc) as tc:
        with tc.tile_pool(name="persist", bufs=1) as pp:
            qT = pp.tile([128, NFT, L], CDT)
            kT = pp.tile([128, NFT, L], CDT)
            vsb = pp.tile([128, NM, VROW], CDT)
            ctxT = pp.tile([128, NFT, L], CDT)
            wo_sb = pp.tile([128, NFT, D], CDT)
            bq_sb = pp.tile([128, NFT], F32)
            bk_sb = pp.tile([128, NFT], F32)
            bvb_sb = pp.tile([128, VROW], F32)
            warm = pp.tile([128, 512], CDT)
            actw = pp.tile([1, 16], F32)

            nc.vector.memset(warm[:], 0.25)
            # preload the exp ACT table set during the DMA-wait window
            nc.scalar.activation(actw[:], warm[0:1, 0:16], EXP, scale=SCALE)

            # small/late-needed tensors via the GpSimd DGE; projection inputs
            # on the Sync DGE so the first matmuls unblock fastest
            nc.gpsimd.dma_start(bq_sb[:], bq_d[:])
            nc.gpsimd.dma_start(bk_sb[:], bk_d[:])
            nc.gpsimd.dma_start(bvb_sb[:], bvb_d[:])
            nc.gpsimd.dma_start(
                wo_sb[:], wo_d.rearrange("(n p) f -> p n f", p=128)
            )

            with tc.tile_pool(name="stageV", bufs=1) as sv:
                wv_sb = sv.tile([128, ND, VROW], CDT)
                xv_sb = sv.tile([128, ND, L], CDT)
                wv_r = wv_d.rearrange("(n p) f -> p n f", p=128)
                xv_r = xvT_d.rearrange("(n p) l -> p n l", p=128)
                for d in range(ND):
                    nc.gpsimd.dma_start(wv_sb[:, d, :], wv_r[:, d, :])
                for d in range(ND):
                    nc.gpsimd.dma_start(xv_sb[:, d, :], xv_r[:, d, :])

                with tc.tile_pool(name="stageQK", bufs=1) as sq:
                    wk_sb = sq.tile([128, ND, FPC], CDT)
                    xk_sb = sq.tile([128, ND, L], CDT)
                    wq_sb = sq.tile([128, ND, FPC], CDT)
                    xq_sb = sq.tile([128, ND, L], CDT)
                    xq_r = xqT_d.rearrange("(n p) l -> p n l", p=128)
                    xk_r = xkT_d.rearrange("(n p) l -> p n l", p=128)
                    wq_r = wq_d.rearrange("(n p) f -> p n f", p=128)
                    wk_r = wk_d.rearrange("(n p) f -> p n f", p=128)
                    for d in range(ND):
                        nc.sync.dma_start(wk_sb[:, d, :], wk_r[:, d, :])
                        nc.sync.dma_start(xk_sb[:, d, :], xk_r[:, d, :])
                    for d in range(ND):
                        nc.sync.dma_start(wq_sb[:, d, :], wq_r[:, d, :])
                        nc.sync.dma_start(xq_sb[:, d, :], xq_r[:, d, :])

                    with tc.tile_pool(name="psW", bufs=1, space="PSUM") as psW:
                        wps = psW.tile([128, 512], F32)
                        # HAM warmup + cover DGE spin-up before inputs land
                        for i in range(22):
                            nc.tensor.matmul(
                                wps[:], warm[:, 0:128], warm[:],
                                start=True, stop=True, skip_group_check=True,
                            )
                        with tc.tile_pool(name="psA", bufs=4, space="PSUM") as psA:
                            def proj_qk(x_sb, w_sb, b_sb, dstT, ti):
                                for ft in range(NFT):
                                    for ch in range(4):
                                        ps = psA.tile(
                                            [128, 512], F32, tag="pjk",
                                            name=f"pjk_{ti}_{ft}_{ch}",
                                        )
                                        for d in range(ND):
                                            nc.tensor.matmul(
                                                ps[:],
                                                w_sb[:, d, ft * 128:(ft + 1) * 128],
                                                x_sb[:, d, ch * 512:(ch + 1) * 512],
                                                start=(d == 0),
                                                stop=(d == ND - 1),
                                            )
                                        nc.vector.tensor_scalar_add(
                                            dstT[:, ft, ch * 512:(ch + 1) * 512],
                                            ps[:],
                                            b_sb[:, ft:ft + 1],
                                        )

                            proj_qk(xk_sb, wk_sb, bk_sb, kT, 0)
                            proj_qk(xq_sb, wq_sb, bq_sb, qT, 1)

                # ---- attention pipeline ----
                with (
                    tc.tile_pool(name="probs", bufs=20) as pb,
                    tc.tile_pool(name="smalls", bufs=2) as sm,
                    tc.tile_pool(name="psS", bufs=2, space="PSUM") as psS,
                    tc.tile_pool(name="psC", bufs=2, space="PSUM") as psC,
                ):
                    probs = {}
                    chains = {}

                    def score_pair(si, m):
                        qb, hp = HB[si]
                        sc = psS.tile(
                            [128, 2, 512], F32, tag="sc", name=f"sc_{si}_{m}"
                        )
                        for hi in range(2):
                            po = hi * 64
                            nc.tensor.matmul(
                                sc[:, hi, :],
                                kT[po:po + 64, hp, m * 128:(m + 1) * 128],
                                qT[po:po + 64, hp, qb * 512:(qb + 1) * 512],
                                start=True,
                                stop=True,
                            )
                        pr = pb.tile(
                            [128, 2, 512], CDT, tag="pr", name=f"pr_{si}_{m}"
                        )
                        nc.scalar.activation(pr[:], sc[:], EXP, scale=SCALE)
                        probs[(si, m)] = pr

                    def start_chains(si):
                        qb, hp = HB[si]
                        for hi in range(2):
                            chains[(si, hi)] = psC.tile(
                                [VW, 512], F32, tag="ch", name=f"ch_{si}_{hi}"
                            )

                    def chain_m(si, m):
                        qb, hp = HB[si]
                        pr = probs[(si, m)]
                        for hi in range(2):
                            h = 2 * hp + hi
                            nc.tensor.matmul(
                                chains[(si, hi)][:],
                                vsb[:, m, h * VW:(h + 1) * VW],
                                pr[:, hi, :],
                                start=(m == 0),
                                stop=(m == NM - 1),
                            )
                        del probs[(si, m)]

                    def normalize(si):
                        qb, hp = HB[si]
                        for hi in range(2):
                            ch = chains.pop((si, hi))
                            den = sm.tile(
                                [1, 512], F32, tag="den", name=f"den_{si}_{hi}"
                            )
                            # reciprocal_approx_fast needs base partition 0
                            nc.vector.tensor_copy(den[:], ch[64:65, :])
                            rec = sm.tile(
                                [1, 512], F32, tag="rec", name=f"rec_{si}_{hi}"
                            )
                            nc.vector.reciprocal_approx_fast(rec[:], den[:])
                            rbb = sm.tile(
                                [64, 512], F32, tag="rbb", name=f"rbb_{si}_{hi}"
                            )
                            nc.gpsimd.partition_broadcast(
                                rbb[:], rec[:], channels=64
                            )
                            po = hi * 64
                            nc.vector.tensor_mul(
                                ctxT[po:po + 64, hp, qb * 512:(qb + 1) * 512],
                                ch[0:64, :],
                                rbb[:],
                            )

                    def outproj(qb, psX):
                        for ft8 in range(D // 128):
                            ops = psX.tile(
                                [128, 512], F32, tag="op", name=f"op_{qb}_{ft8}"
                            )
                            for d2 in range(NFT):
                                nc.tensor.matmul(
                                    ops[:],
                                    wo_sb[:, d2, ft8 * 128:(ft8 + 1) * 128],
                                    ctxT[:, d2, qb * 512:(qb + 1) * 512],
                                    start=(d2 == 0),
                                    stop=(d2 == NFT - 1),
                                )
                            st = sm.tile(
                                [128, 512], CDT, tag="ost", bufs=4,
                                name=f"st_{qb}_{ft8}",
                            )
                            nc.vector.tensor_copy(st[:], ops[:])
                            nc.gpsimd.dma_start(
                                out_d[
                                    ft8 * 128:(ft8 + 1) * 128,
                                    qb * 512:(qb + 1) * 512,
                                ],
                                st[:],
                            )

                    with tc.tile_pool(name="psV", bufs=2, space="PSUM") as psV:
                        # v projection interleaved with block-0 scores: ScalarE
                        # starts exp'ing while the PE finishes phase A
                        for m in range(NM):
                            score_pair(0, m)
                            ps = psV.tile(
                                [128, 512], F32, tag="pv", name=f"pv_{m}"
                            )
                            for d in range(ND):
                                nc.tensor.matmul(
                                    ps[:, 0:VROW],
                                    xv_sb[:, d, m * 128:(m + 1) * 128],
                                    wv_sb[:, d, :],
                                    start=(d == 0),
                                    stop=(d == ND - 1),
                                )
                            nc.vector.tensor_add(
                                vsb[:, m, :], ps[:, 0:VROW], bvb_sb[:]
                            )
                        # block 0 chains + block 1 scores
                        start_chains(0)
                        for m in range(NM):
                            chain_m(0, m)
                            score_pair(1, m)
                        normalize(0)

                    with tc.tile_pool(name="psX", bufs=2, space="PSUM") as psX:
                        for i in range(2, len(HB) + 1):
                            prev = i - 1
                            start_chains(prev)
                            for m in range(NM):
                                chain_m(prev, m)
                                if i < len(HB):
                                    score_pair(i, m)
                            normalize(prev)
                            qb, hp = HB[prev]
                            if hp == 1:
                                outproj(qb, psX)
    nc.compile()
    return nc


def make_in_maps(Q, K, V, Wq, bq, Wk, bk, Wv, bv, Wo, bo):
    Q = np.asarray(Q, np.float32)
    K = np.asarray(K, np.float32)
    V = np.asarray(V, np.float32)
    xqT = [np.ascontiguousarray(Q[b].T).astype(NP_CDT) for b in range(B)]
    xkT = [np.ascontiguousarray(K[b].T).astype(NP_CDT) for b in range(B)]
    xvT = [np.ascontiguousarray(V[b].T).astype(NP_CDT) for b in range(B)]
    in_maps = []
    for c in range(NCORES):
        b, g = divmod(c, HPC)
        fs = slice(g * FPC, (g + 1) * FPC)
        wqT = np.ascontiguousarray(np.asarray(Wq, np.float32)[fs, :].T).astype(NP_CDT)
        wkT = np.ascontiguousarray(np.asarray(Wk, np.float32)[fs, :].T).astype(NP_CDT)
        # v weights: per-head [64 cols | zero col], bias bcast carries the 1.0
        wv_blk = np.zeros((D, VROW), np.float32)
        bv_blk = np.zeros((VROW,), np.float32)
        wv_slc = np.asarray(Wv, np.float32)[fs, :].T  # [D, 256]
        bv_slc = np.asarray(bv, np.float32)[fs]
        for h in range(HPC):
            wv_blk[:, h * VW : h * VW + DH] = wv_slc[:, h * DH : (h + 1) * DH]
            bv_blk[h * VW : h * VW + DH] = bv_slc[h * DH : (h + 1) * DH]
            bv_blk[h * VW + DH] = 1.0
        woT = np.ascontiguousarray(np.asarray(Wo, np.float32)[:, fs].T).astype(NP_CDT)
        bq2 = np.ascontiguousarray(
            np.asarray(bq, np.float32)[fs].reshape(NFT, 128).T
        )
        bk2 = np.ascontiguousarray(
            np.asarray(bk, np.float32)[fs].reshape(NFT, 128).T
        )
        in_maps.append(
            {
                "xqT": xqT[b],
                "xkT": xkT[b],
                "xvT": xvT[b],
                "wqT": wqT,
                "wkT": wkT,
                "wvT": wv_blk.astype(NP_CDT),
                "woT": woT,
                "bq2": bq2,
                "bk2": bk2,
                "bvb": np.broadcast_to(bv_blk, (128, VROW)).copy(),
            }
        )
    return in_maps


def assemble(results, bo):
    out = np.zeros((B, L, D), np.float32)
    for c in range(NCORES):
        b = c // HPC
        out[b] += np.asarray(results[c][OUT_NAME], np.float32).T
    out += np.asarray(bo, np.float32)[None, None, :]
    return out


def kernel(Q, K, V, Wq, bq, Wk, bk, Wv, bv, Wo, bo):
    if "nc" not in _CACHE:
        _CACHE["nc"] = build_nc()
    nc = _CACHE["nc"]
    in_maps = make_in_maps(Q, K, V, Wq, bq, Wk, bk, Wv, bv, Wo, bo)
    res = run_bass_kernel_spmd(nc, in_maps, core_ids=list(range(NCORES)))
    return assemble(res.results, bo)


# revision 4
# speedup vs baseline: 1.3088x; 1.0475x over previous
"""Multi-head attention (B=2, L=2048, D=1024, H=16) on 8 TRN2 NeuronCores.

Sharding: core c handles batch b = c//4 and head group g = c%4 (4 heads,
256 features). No inter-core communication; host sums the 4 per-head-group
output partials per batch and adds bo.

Per-core schedule (engine-balanced software pipeline):
  - warmup matmuls on a memset tile cover the ~9us framework/DMA startup and
    hold the PE HAM clock-gate at 2.4GHz; a tiny exp() preloads the ACT table
    and a dummy partition_broadcast preloads the GpSimd ucode library
  - input DMAs split across the Sync and GpSimd DGEs: xk on Sync (k proj is
    the scores gate), xq on GpSimd, xv halves on both
  - k projection (DMA-paced, d-outer), then q projection for query-block 0
    only; remaining q chunks and the v projection interleave with block-0
    scores so ScalarE starts exp'ing ~32us in
  - attention runs as 8 half-blocks (512 queries x head-pair). Per key tile m:
    the two heads' scores matmuls (K=64) issue back-to-back at PE row groups
    (0,0)/(64,0) so they stream concurrently in the array; one [128,1024] exp
    on ScalarE; two accumulating attn@V chain matmuls (ones-column emits the
    softmax denominators). The sc PSUM ring paces the PE to ScalarE's rate.
  - normalization: DVE reciprocal + GpSimd partition_broadcast + DVE multiply
    (no PE broadcast matmuls); output projection per query block with PSUM
    evacuation alternating between ScalarE and VectorE and output DMAs
    alternating between both DGE queues.
"""

import math
import sys

sys.path.insert(0, "/opt/trn_rl_repo")

import ml_dtypes
import numpy as np

import concourse.bass as bass
import concourse.mybir as mybir
import concourse.tile as tile
from concourse import bacc
from concourse.bass_utils import run_bass_kernel_spmd

B, L, D, H, DH = 2, 2048, 1024, 16, 64
NCORES = 8
HPC = 4                  # heads per core
FPC = HPC * DH           # 256 features per core
ND = D // 128            # 8 contraction tiles
NFT = FPC // 128         # 2 feature tiles for q/k/ctx
NM = L // 128            # 16 key tiles
VW = DH + 1              # 65 = head block width in v (64 feats + ones col)
VROW = HPC * VW          # 260
NQB = 4                  # 512-query blocks
SCALE = 1.0 / math.sqrt(DH)
CDT = mybir.dt.bfloat16
NP_CDT = ml_dtypes.bfloat16
F32 = mybir.dt.float32
EXP = mybir.ActivationFunctionType.Exp
OUT_NAME = "outT"
# half-blocks: (query block, head pair)
HB = [(qb, hp) for qb in range(NQB) for hp in range(2)]

_CACHE = {}


def build_nc():
    nc = bacc.Bacc(
        "TRN2",
        target_bir_lowering=False,
        debug=False,
        enable_asserts=False,
        num_devices=NCORES,
    )
    xqT_d = nc.dram_tensor("xqT", [D, L], CDT, kind="ExternalInput")
    xkT_d = nc.dram_tensor("xkT", [D, L], CDT, kind="ExternalInput")
    xvT_d = nc.dram_tensor("xvT", [D, L], CDT, kind="ExternalInput")
    wq_d = nc.dram_tensor("wqT", [D, FPC], CDT, kind="ExternalInput")
    wk_d = nc.dram_tensor("wkT", [D, FPC], CDT, kind="ExternalInput")
    wv_d = nc.dram_tensor("wvT", [D, VROW], CDT, kind="ExternalInput")
    wo_d = nc.dram_tensor("woT", [FPC, D], CDT, kind="ExternalInput")
    bq_d = nc.dram_tensor("bq2", [128, NFT], F32, kind="ExternalInput")
    bk_d = nc.dram_tensor("bk2", [128, NFT], F32, kind="ExternalInput")
    bvb_d = nc.dram_tensor("bvb", [128, VROW], F32, kind="ExternalInput")
    out_d = nc.dram_tensor(OUT_NAME, [D, L], CDT, kind="ExternalOutput")

    with tile.TileContext(nc) as tc:
        with tc.tile_pool(name="persist", bufs=1) as pp:
            qT = pp.tile([128, NFT, L], CDT)
            kT = pp.tile([128, NFT, L], CDT)
            vsb = pp.tile([128, NM, VROW], CDT)
            ctxT = pp.tile([128, NFT, L], CDT)
            wo_sb = pp.tile([128, NFT, D], CDT)
            bq_sb = pp.tile([128, NFT], F32)
            bk_sb = pp.tile([128, NFT], F32)
            bvb_sb = pp.tile([128, VROW], F32)
            warm = pp.tile([128, 512], CDT)
            actw = pp.tile([1, 16], F32)
            bcw_in = pp.tile([1, 16], F32)
            bcw = pp.tile([64, 16], F32)

            nc.vector.memset(warm[:], 0.25)
            nc.vector.memset(bcw_in[:], 1.0)
            # preload the exp ACT table set during the DMA-wait window
            nc.scalar.activation(actw[:], warm[0:1, 0:16], EXP, scale=SCALE)
            # preload the GpSimd ucode library (LOAD_LIB costs ~7us; take it
            # during startup instead of at the first normalize)
            nc.gpsimd.partition_broadcast(bcw[:], bcw_in[:], channels=64)

            # GpSimd DGE: small tensors, then xq, then the xv back half.
            nc.gpsimd.dma_start(bq_sb[:], bq_d[:])
            nc.gpsimd.dma_start(bk_sb[:], bk_d[:])
            nc.gpsimd.dma_start(bvb_sb[:], bvb_d[:])
            nc.gpsimd.dma_start(
                wo_sb[:], wo_d.rearrange("(n p) f -> p n f", p=128)
            )

            with tc.tile_pool(name="stageV", bufs=1) as sv:
                wv_sb = sv.tile([128, ND, VROW], CDT)
                xv_sb = sv.tile([128, ND, L], CDT)
                wv_r = wv_d.rearrange("(n p) f -> p n f", p=128)
                xv_r = xvT_d.rearrange("(n p) l -> p n l", p=128)
                for d in range(ND):
                    nc.gpsimd.dma_start(wv_sb[:, d, :], wv_r[:, d, :])

                with tc.tile_pool(name="stageQK", bufs=1) as sq:
                    wk_sb = sq.tile([128, ND, FPC], CDT)
                    xk_sb = sq.tile([128, ND, L], CDT)
                    wq_sb = sq.tile([128, ND, FPC], CDT)
                    xq_sb = sq.tile([128, ND, L], CDT)
                    xq_r = xqT_d.rearrange("(n p) l -> p n l", p=128)
                    xk_r = xkT_d.rearrange("(n p) l -> p n l", p=128)
                    wq_r = wq_d.rearrange("(n p) f -> p n f", p=128)
                    wk_r = wk_d.rearrange("(n p) f -> p n f", p=128)
                    # Sync DGE: k inputs first (scores gate on the full kT)
                    for d in range(ND):
                        nc.sync.dma_start(wk_sb[:, d, :], wk_r[:, d, :])
                        nc.sync.dma_start(xk_sb[:, d, :], xk_r[:, d, :])
                    # GpSimd DGE: q inputs in parallel with xk
                    for d in range(ND):
                        nc.gpsimd.dma_start(wq_sb[:, d, :], wq_r[:, d, :])
                        nc.gpsimd.dma_start(xq_sb[:, d, :], xq_r[:, d, :])
                    # xv split across both queues behind the above
                    for d in range(ND):
                        eng = nc.sync if d < ND // 2 else nc.gpsimd
                        eng.dma_start(xv_sb[:, d, :], xv_r[:, d, :])

                    with tc.tile_pool(name="psW", bufs=1, space="PSUM") as psW:
                        wps = psW.tile([128, 512], F32)
                        # HAM warmup + cover DGE spin-up before inputs land
                        for i in range(22):
                            nc.tensor.matmul(
                                wps[:], warm[:, 0:128], warm[:],
                                start=True, stop=True, skip_group_check=True,
                            )
                        with tc.tile_pool(name="psA", bufs=4, space="PSUM") as psA:
                            # k projection: all 4 query... key chunks, d-outer
                            # so matmuls chase the DMA arrivals
                            for ft in range(NFT):
                                pss = [
                                    psA.tile([128, 512], F32, tag="pjk",
                                             name=f"pk_{ft}_{ch}")
                                    for ch in range(4)
                                ]
                                for d in range(ND):
                                    for ch in range(4):
                                        nc.tensor.matmul(
                                            pss[ch][:],
                                            wk_sb[:, d, ft * 128:(ft + 1) * 128],
                                            xk_sb[:, d, ch * 512:(ch + 1) * 512],
                                            start=(d == 0),
                                            stop=(d == ND - 1),
                                        )
                                for ch in range(4):
                                    nc.vector.tensor_scalar_add(
                                        kT[:, ft, ch * 512:(ch + 1) * 512],
                                        pss[ch][:],
                                        bk_sb[:, ft:ft + 1],
                                    )
                            # q projection, query-block 0 only (d-outer)
                            pss = [
                                psA.tile([128, 512], F32, tag="pjk",
                                         name=f"pq_{ft}_0")
                                for ft in range(NFT)
                            ]
                            for d in range(ND):
                                for ft in range(NFT):
                                    nc.tensor.matmul(
                                        pss[ft][:],
                                        wq_sb[:, d, ft * 128:(ft + 1) * 128],
                                        xq_sb[:, d, 0:512],
                                        start=(d == 0),
                                        stop=(d == ND - 1),
                                    )
                            for ft in range(NFT):
                                nc.vector.tensor_scalar_add(
                                    qT[:, ft, 0:512],
                                    pss[ft][:],
                                    bq_sb[:, ft:ft + 1],
                                )

                    # ---- attention pipeline ----
                    with (
                        tc.tile_pool(name="probs", bufs=20) as pb,
                        tc.tile_pool(name="smalls", bufs=2) as sm,
                        tc.tile_pool(name="psS", bufs=2, space="PSUM") as psS,
                        tc.tile_pool(name="psC", bufs=2, space="PSUM") as psC,
                    ):
                        probs = {}
                        chains = {}

                        def score_pair(si, m):
                            qb, hp = HB[si]
                            sc = psS.tile(
                                [128, 2, 512], F32, tag="sc", name=f"sc_{si}_{m}"
                            )
                            for hi in range(2):
                                po = hi * 64
                                nc.tensor.matmul(
                                    sc[:, hi, :],
                                    kT[po:po + 64, hp, m * 128:(m + 1) * 128],
                                    qT[po:po + 64, hp, qb * 512:(qb + 1) * 512],
                                    start=True,
                                    stop=True,
                                )
                            pr = pb.tile(
                                [128, 2, 512], CDT, tag="pr", name=f"pr_{si}_{m}"
                            )
                            nc.scalar.activation(pr[:], sc[:], EXP, scale=SCALE)
                            probs[(si, m)] = pr

                        def start_chains(si):
                            for hi in range(2):
                                chains[(si, hi)] = psC.tile(
                                    [VW, 512], F32, tag="ch", name=f"ch_{si}_{hi}"
                                )

                        def chain_m(si, m):
                            qb, hp = HB[si]
                            pr = probs[(si, m)]
                            for hi in range(2):
                                h = 2 * hp + hi
                                nc.tensor.matmul(
                                    chains[(si, hi)][:],
                                    vsb[:, m, h * VW:(h + 1) * VW],
                                    pr[:, hi, :],
                                    start=(m == 0),
                                    stop=(m == NM - 1),
                                )
                            del probs[(si, m)]

                        def normalize(si):
                            qb, hp = HB[si]
                            for hi in range(2):
                                ch = chains.pop((si, hi))
                                den = sm.tile(
                                    [1, 512], F32, tag="den", name=f"den_{si}_{hi}"
                                )
                                # reciprocal_approx_fast needs base partition 0
                                nc.vector.tensor_copy(den[:], ch[64:65, :])
                                rec = sm.tile(
                                    [1, 512], F32, tag="rec", name=f"rec_{si}_{hi}"
                                )
                                nc.vector.reciprocal_approx_fast(rec[:], den[:])
                                rbb = sm.tile(
                                    [64, 512], F32, tag="rbb", name=f"rbb_{si}_{hi}"
                                )
                                nc.gpsimd.partition_broadcast(
                                    rbb[:], rec[:], channels=64
                                )
                                po = hi * 64
                                nc.vector.tensor_mul(
                                    ctxT[po:po + 64, hp, qb * 512:(qb + 1) * 512],
                                    ch[0:64, :],
                                    rbb[:],
                                )

                        def outproj(qb, psX):
                            for ft8 in range(D // 128):
                                ops = psX.tile(
                                    [128, 512], F32, tag="op", name=f"op_{qb}_{ft8}"
                                )
                                for d2 in range(NFT):
                                    nc.tensor.matmul(
                                        ops[:],
                                        wo_sb[:, d2, ft8 * 128:(ft8 + 1) * 128],
                                        ctxT[:, d2, qb * 512:(qb + 1) * 512],
                                        start=(d2 == 0),
                                        stop=(d2 == NFT - 1),
                                    )
                                st = sm.tile(
                                    [128, 512], CDT, tag="ost", bufs=4,
                                    name=f"st_{qb}_{ft8}",
                                )
                                # alternate evacuation engine and DMA queue so
                                # the MM->copy->DMA pipeline double-streams
                                if ft8 % 2 == 0:
                                    nc.vector.tensor_copy(st[:], ops[:])
                                else:
                                    nc.scalar.copy(st[:], ops[:])
                                dma_eng = nc.gpsimd if ft8 % 2 == 0 else nc.sync
                                dma_eng.dma_start(
                                    out_d[
                                        ft8 * 128:(ft8 + 1) * 128,
                                        qb * 512:(qb + 1) * 512,
                                    ],
                                    st[:],
                                )

                        with tc.tile_pool(name="psV", bufs=2, space="PSUM") as psV:
                            # deferred work units fed into the block-0 stretch:
                            # q projections for blocks 1-3, then v projection
                            def gen_qrest():
                                # q projection chunks 1..3: per (ch, ft) an
                                # 8-matmul accumulation chain + bias add
                                for ch in range(1, 4):
                                    for ft in range(NFT):
                                        ps = psV.tile(
                                            [128, 512], F32, tag="pv",
                                            name=f"pq_{ch}_{ft}",
                                        )
                                        for d in range(ND):
                                            yield lambda ps=ps, d=d, ft=ft, ch=ch: \
                                                nc.tensor.matmul(
                                                    ps[:],
                                                    wq_sb[:, d, ft * 128:(ft + 1) * 128],
                                                    xq_sb[:, d, ch * 512:(ch + 1) * 512],
                                                    start=(d == 0),
                                                    stop=(d == ND - 1),
                                                )
                                        yield lambda ps=ps, ft=ft, ch=ch: \
                                            nc.vector.tensor_scalar_add(
                                                qT[:, ft, ch * 512:(ch + 1) * 512],
                                                ps[:],
                                                bq_sb[:, ft:ft + 1],
                                            )

                            def gen_v():
                                for kt in range(NM):
                                    ps = psV.tile(
                                        [128, 512], F32, tag="pv",
                                        name=f"pv_{kt}",
                                    )
                                    for d in range(ND):
                                        yield lambda ps=ps, d=d, kt=kt: \
                                            nc.tensor.matmul(
                                                ps[:, 0:VROW],
                                                xv_sb[:, d, kt * 128:(kt + 1) * 128],
                                                wv_sb[:, d, :],
                                                start=(d == 0),
                                                stop=(d == ND - 1),
                                            )
                                    yield lambda ps=ps, kt=kt: \
                                        nc.vector.tensor_add(
                                            vsb[:, kt, :], ps[:, 0:VROW],
                                            bvb_sb[:],
                                        )

                            def chaingen(*gens):
                                for g in gens:
                                    yield from g

                            filler = chaingen(gen_qrest(), gen_v())

                            def emit_filler(n):
                                for _ in range(n):
                                    op = next(filler, None)
                                    if op is None:
                                        return
                                    op()

                            # stretch: block-0 scores paced by ScalarE, with
                            # deferred q/v projection as PE filler
                            for m in range(NM):
                                score_pair(0, m)
                                emit_filler(9)
                            # block 0 chains + block 1 scores + leftover filler
                            start_chains(0)
                            for m in range(NM):
                                chain_m(0, m)
                                score_pair(1, m)
                                emit_filler(4)
                            emit_filler(1000)
                            normalize(0)

                        with tc.tile_pool(name="psX", bufs=2, space="PSUM") as psX:
                            for i in range(2, len(HB) + 1):
                                prev = i - 1
                                start_chains(prev)
                                for m in range(NM):
                                    chain_m(prev, m)
                                    if i < len(HB):
                                        score_pair(i, m)
                                normalize(prev)
                                qb, hp = HB[prev]
                                if hp == 1:
                                    outproj(qb, psX)
    nc.compile()
    return nc


def make_in_maps(Q, K, V, Wq, bq, Wk, bk, Wv, bv, Wo, bo):
    Q = np.asarray(Q, np.float32)
    K = np.asarray(K, np.float32)
    V = np.asarray(V, np.float32)
    xqT = [np.ascontiguousarray(Q[b].T).astype(NP_CDT) for b in range(B)]
    xkT = [np.ascontiguousarray(K[b].T).astype(NP_CDT) for b in range(B)]
    xvT = [np.ascontiguousarray(V[b].T).astype(NP_CDT) for b in range(B)]
    in_maps = []
    for c in range(NCORES):
        b, g = divmod(c, HPC)
        fs = slice(g * FPC, (g + 1) * FPC)
        wqT = np.ascontiguousarray(np.asarray(Wq, np.float32)[fs, :].T).astype(NP_CDT)
        wkT = np.ascontiguousarray(np.asarray(Wk, np.float32)[fs, :].T).astype(NP_CDT)
        # v weights: per-head [64 cols | zero col], bias bcast carries the 1.0
        wv_blk = np.zeros((D, VROW), np.float32)
        bv_blk = np.zeros((VROW,), np.float32)
        wv_slc = np.asarray(Wv, np.float32)[fs, :].T  # [D, 256]
        bv_slc = np.asarray(bv, np.float32)[fs]
        for h in range(HPC):
            wv_blk[:, h * VW : h * VW + DH] = wv_slc[:, h * DH : (h + 1) * DH]
            bv_blk[h * VW : h * VW + DH] = bv_slc[h * DH : (h + 1) * DH]
            bv_blk[h * VW + DH] = 1.0
        woT = np.ascontiguousarray(np.asarray(Wo, np.float32)[:, fs].T).astype(NP_CDT)
        bq2 = np.ascontiguousarray(
            np.asarray(bq, np.float32)[fs].reshape(NFT, 128).T
        )
        bk2 = np.ascontiguousarray(
            np.asarray(bk, np.float32)[fs].reshape(NFT, 128).T
        )
        in_maps.append(
            {
                "xqT": xqT[b],
                "xkT": xkT[b],
                "xvT": xvT[b],
                "wqT": wqT,
                "wkT": wkT,
                "wvT": wv_blk.astype(NP_CDT),
                "woT": woT,
                "bq2": bq2,
                "bk2": bk2,
                "bvb": np.broadcast_to(bv_blk, (128, VROW)).copy(),
            }
        )
    return in_maps


def assemble(results, bo):
    out = np.zeros((B, L, D), np.float32)
    for c in range(NCORES):
        b = c // HPC
        out[b] += np.asarray(results[c][OUT_NAME], np.float32).T
    out += np.asarray(bo, np.float32)[None, None, :]
    return out


def kernel(Q, K, V, Wq, bq, Wk, bk, Wv, bv, Wo, bo):
    if "nc" not in _CACHE:
        _CACHE["nc"] = build_nc()
    nc = _CACHE["nc"]
    in_maps = make_in_maps(Q, K, V, Wq, bq, Wk, bk, Wv, bv, Wo, bo)
    res = run_bass_kernel_spmd(nc, in_maps, core_ids=list(range(NCORES)))
    return assemble(res.results, bo)


# revision 7
# speedup vs baseline: 1.4169x; 1.0826x over previous
"""Multi-head attention (B=2, L=2048, D=1024, H=16) on 8 TRN2 NeuronCores.

Sharding: core c handles batch b = c//4 and head group g = c%4 (4 heads,
256 features). No inter-core communication; host sums the 4 per-head-group
output partials per batch and adds bo.

Per-core schedule (engine-balanced software pipeline):
  - warmup matmuls on a memset tile cover the ~9us framework/DMA startup and
    hold the PE HAM clock-gate at 2.4GHz; a tiny exp() preloads the ACT table
    and a dummy partition_broadcast preloads the GpSimd ucode library
  - input DMAs split across the Sync and GpSimd DGEs: xk on Sync (k proj is
    the scores gate), xq on GpSimd, xv halves on both
  - k projection (DMA-paced, d-outer), then q projection for query-block 0
    only; remaining q chunks and the v projection interleave with block-0
    scores so ScalarE starts exp'ing ~32us in
  - attention runs as 8 half-blocks (512 queries x head-pair). Per key tile m:
    the two heads' scores matmuls (K=64) issue back-to-back at PE row groups
    (0,0)/(64,0) so they stream concurrently in the array; one [128,1024] exp
    on ScalarE; two accumulating attn@V chain matmuls (ones-column emits the
    softmax denominators). The sc PSUM ring paces the PE to ScalarE's rate.
  - normalization: DVE reciprocal + GpSimd partition_broadcast + DVE multiply
    (no PE broadcast matmuls); output projection per query block with PSUM
    evacuation alternating between ScalarE and VectorE and output DMAs
    alternating between both DGE queues.
"""

import math
import sys

sys.path.insert(0, "/opt/trn_rl_repo")

import ml_dtypes
import numpy as np

import concourse.bass as bass
import concourse.mybir as mybir
import concourse.tile as tile
from concourse import bacc
from concourse.bass_utils import run_bass_kernel_spmd

B, L, D, H, DH = 2, 2048, 1024, 16, 64
NCORES = 8
HPC = 4                  # heads per core
FPC = HPC * DH           # 256 features per core
ND = D // 128            # 8 contraction tiles
NFT = FPC // 128         # 2 feature tiles for q/k/ctx
NM = L // 128            # 16 key tiles
VW = DH + 1              # 65 = head block width in v (64 feats + ones col)
VROW = HPC * VW          # 260
NQB = 4                  # 512-query blocks
SCALE = 1.0 / math.sqrt(DH)
CDT = mybir.dt.bfloat16
NP_CDT = ml_dtypes.bfloat16
F32 = mybir.dt.float32
EXP = mybir.ActivationFunctionType.Exp
OUT_NAME = "outT"
# half-blocks: (query block, head pair)
HB = [(qb, hp) for qb in range(NQB) for hp in range(2)]

_CACHE = {}


def build_nc():
    nc = bacc.Bacc(
        "TRN2",
        target_bir_lowering=False,
        debug=False,
        enable_asserts=False,
        num_devices=NCORES,
    )
    xqT_d = nc.dram_tensor("xqT", [D, L], CDT, kind="ExternalInput")
    xkT_d = nc.dram_tensor("xkT", [D, L], CDT, kind="ExternalInput")
    xvT_d = nc.dram_tensor("xvT", [D, L], CDT, kind="ExternalInput")
    wq_d = nc.dram_tensor("wqT", [D, FPC], CDT, kind="ExternalInput")
    wk_d = nc.dram_tensor("wkT", [D, FPC], CDT, kind="ExternalInput")
    wv_d = nc.dram_tensor("wvT", [D, VROW], CDT, kind="ExternalInput")
    wo_d = nc.dram_tensor("woT", [FPC, D], CDT, kind="ExternalInput")
    bq_d = nc.dram_tensor("bq2", [128, NFT], F32, kind="ExternalInput")
    bk_d = nc.dram_tensor("bk2", [128, NFT], F32, kind="ExternalInput")
    bvb_d = nc.dram_tensor("bvb", [128, VROW], F32, kind="ExternalInput")
    out_d = nc.dram_tensor(OUT_NAME, [D, L], CDT, kind="ExternalOutput")

    with tile.TileContext(nc) as tc:
        with tc.tile_pool(name="persist", bufs=1) as pp:
            qT = pp.tile([128, NFT, L], CDT)
            kT = pp.tile([128, NFT, L], CDT)
            vsb = pp.tile([128, NM, VROW], CDT)
            ctxT = pp.tile([128, NFT, L], CDT)
            wo_sb = pp.tile([128, NFT, D], CDT)
            bq_sb = pp.tile([128, NFT], F32)
            bk_sb = pp.tile([128, NFT], F32)
            bvb_sb = pp.tile([128, VROW], F32)
            warm = pp.tile([128, 512], CDT)
            actw = pp.tile([1, 16], F32)
            bcw_in = pp.tile([1, 16], F32)
            bcw = pp.tile([64, 16], F32)

            nc.vector.memset(warm[:], 0.25)
            nc.vector.memset(bcw_in[:], 1.0)
            # preload the exp ACT table set during the DMA-wait window
            nc.scalar.activation(actw[:], warm[0:1, 0:16], EXP, scale=SCALE)

            with tc.tile_pool(name="stageV", bufs=1) as sv:
                wv_sb = sv.tile([128, ND, VROW], CDT)
                xv_sb = sv.tile([128, ND, L], CDT)
                wv_r = wv_d.rearrange("(n p) f -> p n f", p=128)
                xv_r = xvT_d.rearrange("(n p) l -> p n l", p=128)

                with tc.tile_pool(name="stageQK", bufs=1) as sq:
                    wk_sb = sq.tile([128, ND, FPC], CDT)
                    xk_sb = sq.tile([128, ND, L], CDT)
                    wq_sb = sq.tile([128, ND, FPC], CDT)
                    xq_sb = sq.tile([128, ND, L], CDT)
                    xq_r = xqT_d.rearrange("(n p) l -> p n l", p=128)
                    xk_r = xkT_d.rearrange("(n p) l -> p n l", p=128)
                    wq_r = wq_d.rearrange("(n p) f -> p n f", p=128)
                    wk_r = wk_d.rearrange("(n p) f -> p n f", p=128)
                    # Both DGE queues share the 16 DMA engines, so arrival
                    # order == issue order by priority: xk (gates scores),
                    # then wq+xq, then wv+xv, then wo. Even d-slices on Sync,
                    # odd on GpSimd.
                    nc.gpsimd.dma_start(bq_sb[:], bq_d[:])
                    nc.gpsimd.dma_start(bk_sb[:], bk_d[:])
                    nc.gpsimd.dma_start(bvb_sb[:], bvb_d[:])
                    for d in range(ND):
                        nc.sync.dma_start(wk_sb[:, d, :], wk_r[:, d, :])
                        eng = nc.sync if d % 2 == 0 else nc.gpsimd
                        eng.dma_start(xk_sb[:, d, :], xk_r[:, d, :])
                    for d in range(ND):
                        nc.gpsimd.dma_start(wq_sb[:, d, :], wq_r[:, d, :])
                    for d in range(ND):
                        eng = nc.sync if d % 2 == 0 else nc.gpsimd
                        eng.dma_start(xq_sb[:, d, :], xq_r[:, d, :])
                    # preload the GpSimd ucode library here (LOAD_LIB costs
                    # ~7us of GpSimd queue time; it must precede the first
                    # normalize but not delay the xk/xq descriptors)
                    nc.gpsimd.partition_broadcast(bcw[:], bcw_in[:], channels=64)
                    for d in range(ND):
                        nc.sync.dma_start(wv_sb[:, d, :], wv_r[:, d, :])
                    for d in range(ND):
                        eng = nc.sync if d % 2 == 0 else nc.gpsimd
                        eng.dma_start(xv_sb[:, d, :], xv_r[:, d, :])
                    nc.gpsimd.dma_start(
                        wo_sb[:], wo_d.rearrange("(n p) f -> p n f", p=128)
                    )

                    with tc.tile_pool(name="psW", bufs=1, space="PSUM") as psW:
                        wps = psW.tile([128, 512], F32)
                        # HAM warmup + cover DGE spin-up before inputs land
                        for i in range(22):
                            nc.tensor.matmul(
                                wps[:], warm[:, 0:128], warm[:],
                                start=True, stop=True, skip_group_check=True,
                            )
                        with tc.tile_pool(name="psA", bufs=4, space="PSUM") as psA:
                            # k projection: all 4 query... key chunks, d-outer
                            # so matmuls chase the DMA arrivals
                            for ft in range(NFT):
                                pss = [
                                    psA.tile([128, 512], F32, tag="pjk",
                                             name=f"pk_{ft}_{ch}")
                                    for ch in range(4)
                                ]
                                for d in range(ND):
                                    for ch in range(4):
                                        nc.tensor.matmul(
                                            pss[ch][:],
                                            wk_sb[:, d, ft * 128:(ft + 1) * 128],
                                            xk_sb[:, d, ch * 512:(ch + 1) * 512],
                                            start=(d == 0),
                                            stop=(d == ND - 1),
                                        )
                                for ch in range(4):
                                    nc.vector.tensor_scalar_add(
                                        kT[:, ft, ch * 512:(ch + 1) * 512],
                                        pss[ch][:],
                                        bk_sb[:, ft:ft + 1],
                                    )
                            # q projection, query-block 0 only (d-outer)
                            pss = [
                                psA.tile([128, 512], F32, tag="pjk",
                                         name=f"pq_{ft}_0")
                                for ft in range(NFT)
                            ]
                            for d in range(ND):
                                for ft in range(NFT):
                                    nc.tensor.matmul(
                                        pss[ft][:],
                                        wq_sb[:, d, ft * 128:(ft + 1) * 128],
                                        xq_sb[:, d, 0:512],
                                        start=(d == 0),
                                        stop=(d == ND - 1),
                                    )
                            for ft in range(NFT):
                                nc.vector.tensor_scalar_add(
                                    qT[:, ft, 0:512],
                                    pss[ft][:],
                                    bq_sb[:, ft:ft + 1],
                                )

                    # ---- attention pipeline ----
                    with (
                        tc.tile_pool(name="probs", bufs=20) as pb,
                        tc.tile_pool(name="smalls", bufs=2) as sm,
                        tc.tile_pool(name="psS", bufs=2, space="PSUM") as psS,
                        tc.tile_pool(name="psC", bufs=2, space="PSUM") as psC,
                    ):
                        probs = {}
                        chains = {}

                        def score_pair(si, m):
                            qb, hp = HB[si]
                            sc = psS.tile(
                                [128, 2, 512], F32, tag="sc", name=f"sc_{si}_{m}"
                            )
                            for hi in range(2):
                                po = hi * 64
                                nc.tensor.matmul(
                                    sc[:, hi, :],
                                    kT[po:po + 64, hp, m * 128:(m + 1) * 128],
                                    qT[po:po + 64, hp, qb * 512:(qb + 1) * 512],
                                    start=True,
                                    stop=True,
                                )
                            pr = pb.tile(
                                [128, 2, 512], CDT, tag="pr", name=f"pr_{si}_{m}"
                            )
                            nc.scalar.activation(pr[:], sc[:], EXP, scale=SCALE)
                            probs[(si, m)] = pr

                        def start_chains(si):
                            for hi in range(2):
                                chains[(si, hi)] = psC.tile(
                                    [VW, 512], F32, tag="ch", name=f"ch_{si}_{hi}"
                                )

                        def chain_m(si, m):
                            qb, hp = HB[si]
                            pr = probs[(si, m)]
                            for hi in range(2):
                                h = 2 * hp + hi
                                nc.tensor.matmul(
                                    chains[(si, hi)][:],
                                    vsb[:, m, h * VW:(h + 1) * VW],
                                    pr[:, hi, :],
                                    start=(m == 0),
                                    stop=(m == NM - 1),
                                )
                            del probs[(si, m)]

                        def normalize(si):
                            qb, hp = HB[si]
                            for hi in range(2):
                                ch = chains.pop((si, hi))
                                den = sm.tile(
                                    [1, 512], F32, tag="den", name=f"den_{si}_{hi}"
                                )
                                # reciprocal_approx_fast needs base partition 0
                                nc.vector.tensor_copy(den[:], ch[64:65, :])
                                rec = sm.tile(
                                    [1, 512], F32, tag="rec", name=f"rec_{si}_{hi}"
                                )
                                nc.vector.reciprocal_approx_fast(rec[:], den[:])
                                rbb = sm.tile(
                                    [64, 512], F32, tag="rbb", name=f"rbb_{si}_{hi}"
                                )
                                nc.gpsimd.partition_broadcast(
                                    rbb[:], rec[:], channels=64
                                )
                                po = hi * 64
                                nc.vector.tensor_mul(
                                    ctxT[po:po + 64, hp, qb * 512:(qb + 1) * 512],
                                    ch[0:64, :],
                                    rbb[:],
                                )

                        pending = []

                        def outproj_unit(qb, ft8, psX):
                            ops = psX.tile(
                                [128, 512], F32, tag="op", name=f"op_{qb}_{ft8}"
                            )
                            for d2 in range(NFT):
                                nc.tensor.matmul(
                                    ops[:],
                                    wo_sb[:, d2, ft8 * 128:(ft8 + 1) * 128],
                                    ctxT[:, d2, qb * 512:(qb + 1) * 512],
                                    start=(d2 == 0),
                                    stop=(d2 == NFT - 1),
                                )
                            st = sm.tile(
                                [128, 512], CDT, tag="ost", bufs=4,
                                name=f"st_{qb}_{ft8}",
                            )
                            # alternate evacuation engine and DMA queue so
                            # the MM->copy->DMA pipeline double-streams
                            if ft8 % 2 == 0:
                                nc.vector.tensor_copy(st[:], ops[:])
                            else:
                                nc.scalar.copy(st[:], ops[:])
                            dma_eng = nc.gpsimd if ft8 % 2 == 0 else nc.sync
                            dma_eng.dma_start(
                                out_d[
                                    ft8 * 128:(ft8 + 1) * 128,
                                    qb * 512:(qb + 1) * 512,
                                ],
                                st[:],
                            )

                        with tc.tile_pool(name="psV", bufs=2, space="PSUM") as psV:
                            # deferred work units fed into the block-0 stretch:
                            # q projections for blocks 1-3, then v projection
                            def gen_qrest():
                                # q projection chunks 1..3: per (ch, ft) an
                                # 8-matmul accumulation chain + bias add
                                for ch in range(1, 4):
                                    for ft in range(NFT):
                                        ps = psV.tile(
                                            [128, 512], F32, tag="pv",
                                            name=f"pq_{ch}_{ft}",
                                        )
                                        for d in range(ND):
                                            yield lambda ps=ps, d=d, ft=ft, ch=ch: \
                                                nc.tensor.matmul(
                                                    ps[:],
                                                    wq_sb[:, d, ft * 128:(ft + 1) * 128],
                                                    xq_sb[:, d, ch * 512:(ch + 1) * 512],
                                                    start=(d == 0),
                                                    stop=(d == ND - 1),
                                                )
                                        yield lambda ps=ps, ft=ft, ch=ch: \
                                            nc.vector.tensor_scalar_add(
                                                qT[:, ft, ch * 512:(ch + 1) * 512],
                                                ps[:],
                                                bq_sb[:, ft:ft + 1],
                                            )

                            def gen_v():
                                for kt in range(NM):
                                    ps = psV.tile(
                                        [128, 512], F32, tag="pv",
                                        name=f"pv_{kt}",
                                    )
                                    for d in range(ND):
                                        yield lambda ps=ps, d=d, kt=kt: \
                                            nc.tensor.matmul(
                                                ps[:, 0:VROW],
                                                xv_sb[:, d, kt * 128:(kt + 1) * 128],
                                                wv_sb[:, d, :],
                                                start=(d == 0),
                                                stop=(d == ND - 1),
                                            )
                                    yield lambda ps=ps, kt=kt: \
                                        nc.vector.tensor_add(
                                            vsb[:, kt, :], ps[:, 0:VROW],
                                            bvb_sb[:],
                                        )

                            def chaingen(*gens):
                                for g in gens:
                                    yield from g

                            filler = chaingen(gen_qrest(), gen_v())

                            def emit_filler(n):
                                for _ in range(n):
                                    op = next(filler, None)
                                    if op is None:
                                        return
                                    op()

                            # stretch: block-0 scores paced by ScalarE, with
                            # deferred q/v projection as PE filler
                            for m in range(NM):
                                score_pair(0, m)
                                emit_filler(9)
                            # block 0 chains + block 1 scores + leftover filler
                            start_chains(0)
                            for m in range(NM):
                                chain_m(0, m)
                                score_pair(1, m)
                                emit_filler(4)
                            emit_filler(1000)
                            normalize(0)

                        with tc.tile_pool(name="psX", bufs=2, space="PSUM") as psX:
                            for i in range(2, len(HB) + 1):
                                prev = i - 1
                                start_chains(prev)
                                for m in range(NM):
                                    chain_m(prev, m)
                                    if i < len(HB):
                                        score_pair(i, m)
                                    # spread the previous block's output
                                    # projection into the ScalarE-rate slack
                                    if m % 2 == 0 and pending:
                                        pending.pop(0)()
                                normalize(prev)
                                qb, hp = HB[prev]
                                if hp == 1:
                                    for ft8 in range(D // 128):
                                        pending.append(
                                            lambda qb=qb, ft8=ft8:
                                            outproj_unit(qb, ft8, psX)
                                        )
                            while pending:
                                pending.pop(0)()
    nc.compile()
    return nc


def make_in_maps(Q, K, V, Wq, bq, Wk, bk, Wv, bv, Wo, bo):
    Q = np.asarray(Q, np.float32)
    K = np.asarray(K, np.float32)
    V = np.asarray(V, np.float32)
    xqT = [np.ascontiguousarray(Q[b].T).astype(NP_CDT) for b in range(B)]
    xkT = [np.ascontiguousarray(K[b].T).astype(NP_CDT) for b in range(B)]
    xvT = [np.ascontiguousarray(V[b].T).astype(NP_CDT) for b in range(B)]
    in_maps = []
    for c in range(NCORES):
        b, g = divmod(c, HPC)
        fs = slice(g * FPC, (g + 1) * FPC)
        wqT = np.ascontiguousarray(np.asarray(Wq, np.float32)[fs, :].T).astype(NP_CDT)
        wkT = np.ascontiguousarray(np.asarray(Wk, np.float32)[fs, :].T).astype(NP_CDT)
        # v weights: per-head [64 cols | zero col], bias bcast carries the 1.0
        wv_blk = np.zeros((D, VROW), np.float32)
        bv_blk = np.zeros((VROW,), np.float32)
        wv_slc = np.asarray(Wv, np.float32)[fs, :].T  # [D, 256]
        bv_slc = np.asarray(bv, np.float32)[fs]
        for h in range(HPC):
            wv_blk[:, h * VW : h * VW + DH] = wv_slc[:, h * DH : (h + 1) * DH]
            bv_blk[h * VW : h * VW + DH] = bv_slc[h * DH : (h + 1) * DH]
            bv_blk[h * VW + DH] = 1.0
        woT = np.ascontiguousarray(np.asarray(Wo, np.float32)[:, fs].T).astype(NP_CDT)
        bq2 = np.ascontiguousarray(
            np.asarray(bq, np.float32)[fs].reshape(NFT, 128).T
        )
        bk2 = np.ascontiguousarray(
            np.asarray(bk, np.float32)[fs].reshape(NFT, 128).T
        )
        in_maps.append(
            {
                "xqT": xqT[b],
                "xkT": xkT[b],
                "xvT": xvT[b],
                "wqT": wqT,
                "wkT": wkT,
                "wvT": wv_blk.astype(NP_CDT),
                "woT": woT,
                "bq2": bq2,
                "bk2": bk2,
                "bvb": np.broadcast_to(bv_blk, (128, VROW)).copy(),
            }
        )
    return in_maps


def assemble(results, bo):
    out = np.zeros((B, L, D), np.float32)
    for c in range(NCORES):
        b = c // HPC
        out[b] += np.asarray(results[c][OUT_NAME], np.float32).T
    out += np.asarray(bo, np.float32)[None, None, :]
    return out


def kernel(Q, K, V, Wq, bq, Wk, bk, Wv, bv, Wo, bo):
    if "nc" not in _CACHE:
        _CACHE["nc"] = build_nc()
    nc = _CACHE["nc"]
    in_maps = make_in_maps(Q, K, V, Wq, bq, Wk, bk, Wv, bv, Wo, bo)
    res = run_bass_kernel_spmd(nc, in_maps, core_ids=list(range(NCORES)))
    return assemble(res.results, bo)
